# revision 4
# baseline (speedup 1.0000x reference)
"""MetaPathEncoder as Bass/Tile SPMD kernel on 8 TRN2 cores — v5.

Pair-hybrid sharing: per dst tile, paths are processed in groups of 2.
Each group's gather region is [A | B_p0 | B_p1] where A holds sources used
by BOTH paths of the group (gathered once), B_p holds path-exclusive
sources. Cuts gather indices/bytes ~11% vs per-path dedup while keeping the
streamed-S volume near v3 levels (each path's S spans only A + its own B).
Per-group granularity keeps the gather/compute pipeline fine-grained.
"""
import sys

for _p in ("/opt/trn_rl_repo",):
    if _p not in sys.path:
        sys.path.insert(0, _p)

import numpy as np
import ml_dtypes

import concourse.bass as bass
import concourse.tile as tile
from concourse import bacc, mybir
from concourse.bass_utils import run_bass_kernel_spmd

BF16 = ml_dtypes.bfloat16

N_NODES = 10000
N_PATHS = 4
IN_DIM = 512
OUT_DIM = 512
NCORES = 8
ROWS_PER_CORE = N_NODES // NCORES  # 1250
NTILES = (ROWS_PER_CORE + 127) // 128  # 10
GROUPS = ((0, 1), (2, 3))

SPLIT_BLOCKS = 8
NQUEUES = 4

_program_cache: dict[tuple, object] = {}


def _build_program(struct: tuple):
    """struct[t][gi] = (PA, PB0, PB1) block counts for tile t, group gi."""
    key = (struct, SPLIT_BLOCKS, NQUEUES)
    if key in _program_cache:
        return _program_cache[key]

    dt = mybir.dt
    total_gblk = sum(pa + pb0 + pb1 for tg in struct for (pa, pb0, pb1) in tg)
    total_sblk = sum(2 * pa + pb0 + pb1 for tg in struct for (pa, pb0, pb1) in tg)

    nc = bacc.Bacc(
        "TRN2",
        target_bir_lowering=False,
        debug=False,
        num_devices=NCORES,
        num_swdge_queues=NQUEUES,
    )

    featd = nc.dram_tensor("feat", [N_NODES, IN_DIM], dt.bfloat16, kind="ExternalInput").ap()
    idxd = nc.dram_tensor("idx", [128, total_gblk * 8], dt.int16, kind="ExternalInput").ap()
    sd = nc.dram_tensor("smat", [128, total_sblk * 128], dt.bfloat16, kind="ExternalInput").ap()
    wd = nc.dram_tensor("w", [128, 16 * OUT_DIM], dt.bfloat16, kind="ExternalInput").ap()
    bmd = nc.dram_tensor("bm", [128, OUT_DIM], dt.float32, kind="ExternalInput").ap()
    identd = nc.dram_tensor("identity", [128, 128], dt.bfloat16, kind="ExternalInput").ap()
    outd = nc.dram_tensor("out", [ROWS_PER_CORE, OUT_DIM], dt.float32, kind="ExternalOutput").ap()

    qcounter = [0]

    def next_q():
        q = qcounter[0] % NQUEUES
        qcounter[0] += 1
        return q

    with tile.TileContext(nc) as tc:
        with (
            tc.tile_pool(name="const", bufs=1) as cpool,
            tc.tile_pool(name="g", bufs=4) as gpool,
            tc.tile_pool(name="s", bufs=6) as spool,
            tc.tile_pool(name="hsb", bufs=4) as hsb_pool,
            tc.tile_pool(name="htsb", bufs=4) as htsb_pool,
            tc.tile_pool(name="osb", bufs=2) as osb_pool,
            tc.tile_pool(name="hps", bufs=2, space="PSUM") as hps_pool,
            tc.tile_pool(name="htps", bufs=2, space="PSUM") as htps_pool,
            tc.tile_pool(name="ops", bufs=2, space="PSUM") as ops_pool,
        ):
            idx_sb = cpool.tile([128, total_gblk * 8], dt.int16)
            nc.sync.dma_start(idx_sb[:], idxd[:])
            w_sb = cpool.tile([128, 16 * OUT_DIM], dt.bfloat16)
            nc.sync.dma_start(w_sb[:], wd[:])
            bm_sb = cpool.tile([128, OUT_DIM], dt.float32)
            nc.sync.dma_start(bm_sb[:], bmd[:])
            ident = cpool.tile([128, 128], dt.bfloat16)
            nc.sync.dma_start(ident[:], identd[:])

            goff = 0
            soff = 0
            for t in range(NTILES):
                out_ps = ops_pool.tile([128, OUT_DIM], dt.float32)
                for gi, grp in enumerate(GROUPS):
                    PA, PB0, PB1 = struct[t][gi]
                    TB = PA + PB0 + PB1
                    g = gpool.tile([128, TB, IN_DIM], dt.bfloat16)
                    for j0 in range(0, TB, SPLIT_BLOCKS):
                        j1 = min(j0 + SPLIT_BLOCKS, TB)
                        nb = j1 - j0
                        nc.gpsimd.dma_gather(
                            g[:, j0:j1, :],
                            featd[:],
                            idx_sb[:, (goff + j0) * 8 : (goff + j1) * 8],
                            nb * 128,
                            nb * 128,
                            IN_DIM,
                            single_packet=False,
                            queue_num=next_q(),
                        )
                    for pl, p in enumerate(grp):
                        PBp = PB0 if pl == 0 else PB1
                        bbase = PA if pl == 0 else PA + PB0
                        ng = PA + PBp
                        S = spool.tile([128, ng * 128], dt.bfloat16)
                        nc.sync.dma_start(S[:], sd[:, soff * 128 : (soff + ng) * 128])
                        hp = hps_pool.tile([128, IN_DIM], dt.float32)
                        for k in range(ng):
                            gblk = k if k < PA else bbase + (k - PA)
                            nc.tensor.matmul(
                                hp[:],
                                S[:, k * 128 : (k + 1) * 128],
                                g[:, gblk, :],
                                start=(k == 0),
                                stop=(k == ng - 1),
                            )
                        hs = hsb_pool.tile([128, IN_DIM], dt.bfloat16)
                        nc.scalar.copy(hs[:], hp[:])
                        htp = htps_pool.tile([128, IN_DIM], dt.bfloat16)
                        for cc in range(4):
                            nc.tensor.transpose(
                                htp[:, cc * 128 : (cc + 1) * 128],
                                hs[:, cc * 128 : (cc + 1) * 128],
                                ident[:],
                            )
                        hts = htsb_pool.tile([128, IN_DIM], dt.bfloat16)
                        nc.vector.tensor_copy(hts[:], htp[:])
                        for cc in range(4):
                            nc.tensor.matmul(
                                out_ps[:],
                                hts[:, cc * 128 : (cc + 1) * 128],
                                w_sb[:, (p * 4 + cc) * OUT_DIM : (p * 4 + cc + 1) * OUT_DIM],
                                start=(gi == 0 and pl == 0 and cc == 0),
                                stop=(gi == 1 and pl == 1 and cc == 3),
                            )
                        soff += ng
                    goff += TB
                os_ = osb_pool.tile([128, OUT_DIM], dt.float32)
                nc.vector.tensor_add(os_[:], out_ps[:], bm_sb[:])
                rows = min(128, ROWS_PER_CORE - t * 128)
                nc.sync.dma_start(outd[t * 128 : t * 128 + rows, :], os_[:rows, :])

    nc.compile()
    _program_cache[key] = nc
    return nc


def _prep_host(feat, src, dst, W, b):
    src = np.asarray(src).astype(np.int64)
    dst = np.asarray(dst).astype(np.int64)
    feat = np.asarray(feat, dtype=np.float32)
    W = np.asarray(W, dtype=np.float32)
    b = np.asarray(b, dtype=np.float32)

    feat_bf = feat.astype(BF16)

    Wt = np.empty((128, 16, OUT_DIM), dtype=BF16)
    for p in range(N_PATHS):
        for c in range(4):
            Wt[:, p * 4 + c, :] = W[p, c * 128 : (c + 1) * 128, :].astype(BF16)
    Wt = np.ascontiguousarray(Wt.reshape(128, 16 * OUT_DIM))

    bmean = b.mean(0).astype(np.float32)
    bm_bcast = np.ascontiguousarray(np.broadcast_to(bmean, (128, OUT_DIM)))

    sorted_data = []
    for p in range(N_PATHS):
        s, d = src[p], dst[p]
        deg_out = np.maximum(np.bincount(s, minlength=N_NODES), 1).astype(np.float64)
        deg_in = np.maximum(np.bincount(d, minlength=N_NODES), 1).astype(np.float64)
        ce = (deg_in[d] ** -0.5) * (deg_out[s] ** -0.5) * 0.25
        order = np.argsort(d, kind="stable")
        sorted_data.append((s[order], d[order], ce[order]))

    bounds_lo = np.array(
        [c * ROWS_PER_CORE + t * 128 for c in range(NCORES) for t in range(NTILES)]
    )
    bounds_hi = np.array(
        [
            c * ROWS_PER_CORE + min((t + 1) * 128, ROWS_PER_CORE)
            for c in range(NCORES)
            for t in range(NTILES)
        ]
    )

    ranges = []
    for p in range(N_PATHS):
        ds = sorted_data[p][1]
        a = np.searchsorted(ds, bounds_lo, side="left")
        e = np.searchsorted(ds, bounds_hi, side="left")
        ranges.append((a, e))

    # per (core, tile, group): A (both paths), B per path, plus raw buckets
    layout = [[[None] * len(GROUPS) for _ in range(NTILES)] for _ in range(NCORES)]
    nA = np.zeros((NCORES, NTILES, len(GROUPS)), dtype=np.int64)
    nB = np.zeros((NCORES, NTILES, len(GROUPS), 2), dtype=np.int64)
    for c in range(NCORES):
        for t in range(NTILES):
            lo = c * ROWS_PER_CORE + t * 128
            for gi, grp in enumerate(GROUPS):
                bks = []
                uniqs = []
                for p in grp:
                    a, e = ranges[p][0][c * NTILES + t], ranges[p][1][c * NTILES + t]
                    ss = sorted_data[p][0][a:e]
                    dd = (sorted_data[p][1][a:e] - lo).astype(np.int64)
                    cc = sorted_data[p][2][a:e]
                    bks.append((ss, dd, cc))
                    uniqs.append(np.unique(ss))
                A = np.intersect1d(uniqs[0], uniqs[1], assume_unique=True)
                Bs = [
                    np.setdiff1d(uniqs[0], A, assume_unique=True),
                    np.setdiff1d(uniqs[1], A, assume_unique=True),
                ]
                layout[c][t][gi] = (A, Bs, bks)
                nA[c, t, gi] = len(A)
                nB[c, t, gi, 0] = len(Bs[0])
                nB[c, t, gi, 1] = len(Bs[1])

    struct = []
    for t in range(NTILES):
        tg = []
        for gi in range(len(GROUPS)):
            PA = int(np.ceil(nA[:, t, gi].max() / 128)) or 1
            PB0 = int(np.ceil(nB[:, t, gi, 0].max() / 128)) or 1
            PB1 = int(np.ceil(nB[:, t, gi, 1].max() / 128)) or 1
            tg.append((PA, PB0, PB1))
        struct.append(tuple(tg))
    struct = tuple(struct)

    total_gblk = sum(pa + pb0 + pb1 for tg in struct for (pa, pb0, pb1) in tg)
    total_sblk = sum(2 * pa + pb0 + pb1 for tg in struct for (pa, pb0, pb1) in tg)

    per_core = []
    for c in range(NCORES):
        idxw = np.zeros((128, total_gblk * 8), dtype=np.int16)
        smat = np.zeros((128, total_sblk * 128), dtype=np.float32)
        goff = 0
        soff = 0
        for t in range(NTILES):
            for gi in range(len(GROUPS)):
                PA, PB0, PB1 = struct[t][gi]
                TB = PA + PB0 + PB1
                A, Bs, bks = layout[c][t][gi]
                idx_pad = np.zeros(TB * 128, dtype=np.int16)
                idx_pad[: len(A)] = A
                idx_pad[PA * 128 : PA * 128 + len(Bs[0])] = Bs[0]
                idx_pad[(PA + PB0) * 128 : (PA + PB0) * 128 + len(Bs[1])] = Bs[1]
                w16 = idx_pad.reshape(TB * 8, 16).T
                idxw[:, goff * 8 : (goff + TB) * 8] = np.tile(w16, (8, 1))
                for pl in range(2):
                    PBp = PB0 if pl == 0 else PB1
                    ng = PA + PBp
                    ss, dd, cc = bks[pl]
                    if len(A):
                        posA = np.searchsorted(A, ss)
                        posAc = np.minimum(posA, len(A) - 1)
                        inA = A[posAc] == ss
                    else:
                        posA = np.zeros(len(ss), dtype=np.int64)
                        inA = np.zeros(len(ss), dtype=bool)
                    posB = np.searchsorted(Bs[pl], ss)
                    slot_in_part = np.where(inA, posA % 128, posB % 128)
                    grpi = np.where(inA, posA // 128, PA + posB // 128)
                    np.add.at(
                        smat,
                        (slot_in_part, (soff + grpi) * 128 + dd),
                        cc.astype(np.float32),
                    )
                    soff += ng
                goff += TB
        per_core.append({"idx": idxw, "smat": smat.astype(BF16)})

    shared = {
        "feat": feat_bf,
        "w": Wt,
        "bm": bm_bcast,
        "identity": np.eye(128, dtype=BF16),
    }
    return struct, shared, per_core


def kernel(feat, src, dst, W, b):
    struct, shared, per_core = _prep_host(feat, src, dst, W, b)
    nc = _build_program(struct)
    in_maps = [{**shared, **pc} for pc in per_core]
    res = run_bass_kernel_spmd(nc, in_maps, list(range(NCORES)))
    out = np.concatenate([res.results[c]["out"] for c in range(NCORES)], axis=0)
    return out.astype(np.float32)


if __name__ == "__main__":
    rng = np.random.default_rng(0)
    feat = rng.standard_normal((N_NODES, IN_DIM), dtype=np.float32)
    src = rng.integers(0, N_NODES, (N_PATHS, 160000)).astype(np.int64)
    dst = rng.integers(0, N_NODES, (N_PATHS, 160000)).astype(np.int64)
    W = (rng.standard_normal((N_PATHS, IN_DIM, OUT_DIM), dtype=np.float32) / np.sqrt(IN_DIM)).astype(np.float32)
    b = np.zeros((N_PATHS, OUT_DIM), np.float32)
    out = kernel(feat=feat, src=src, dst=dst, W=W, b=b)
    print("kernel ran, out shape", out.shape, out.dtype)


# revision 5
# speedup vs baseline: 1.0580x; 1.0580x over previous
"""MetaPathEncoder as Bass/Tile SPMD kernel on 8 TRN2 cores — v3.

v2 + knobs:
  - per-path dedup of gathered sources (S rows may hold multiple edges)
  - gather calls split into sub-calls of <= SPLIT_BLOCKS*128 indices
  - gathers striped across SWDGE queues (NQUEUES)
"""
import sys

for _p in ("/opt/trn_rl_repo",):
    if _p not in sys.path:
        sys.path.insert(0, _p)

import numpy as np
import ml_dtypes

import concourse.bass as bass
import concourse.tile as tile
from concourse import bacc, mybir
from concourse.bass_utils import run_bass_kernel_spmd

BF16 = ml_dtypes.bfloat16

N_NODES = 10000
N_PATHS = 4
IN_DIM = 512
OUT_DIM = 512
NCORES = 8
ROWS_PER_CORE = N_NODES // NCORES  # 1250
NTILES = (ROWS_PER_CORE + 127) // 128  # 10
NCALLS = NTILES * N_PATHS  # 40

DEDUP = True
SPLIT_BLOCKS = 8  # max 128-blocks per dma_gather sub-call (1024 idx fits the ring)
NQUEUES = 4

_program_cache: dict[tuple, object] = {}


def _build_program(blocks: tuple):
    key = (blocks, SPLIT_BLOCKS, NQUEUES)
    if key in _program_cache:
        return _program_cache[key]

    dt = mybir.dt
    total_blk = sum(blocks)
    nc = bacc.Bacc(
        "TRN2",
        target_bir_lowering=False,
        debug=False,
        num_devices=NCORES,
        num_swdge_queues=NQUEUES,
    )

    featd = nc.dram_tensor("feat", [N_NODES, IN_DIM], dt.bfloat16, kind="ExternalInput").ap()
    idxd = nc.dram_tensor("idx", [128, total_blk * 8], dt.int16, kind="ExternalInput").ap()
    sd = nc.dram_tensor("smat", [128, total_blk * 128], dt.bfloat16, kind="ExternalInput").ap()
    wd = nc.dram_tensor("w", [128, 16 * OUT_DIM], dt.bfloat16, kind="ExternalInput").ap()
    bmd = nc.dram_tensor("bm", [128, OUT_DIM], dt.float32, kind="ExternalInput").ap()
    identd = nc.dram_tensor("identity", [128, 128], dt.bfloat16, kind="ExternalInput").ap()
    outd = nc.dram_tensor("out", [ROWS_PER_CORE, OUT_DIM], dt.float32, kind="ExternalOutput").ap()

    qcounter = [0]

    def next_q():
        q = qcounter[0] % NQUEUES
        qcounter[0] += 1
        return q

    with tile.TileContext(nc) as tc:
        with (
            tc.tile_pool(name="const", bufs=1) as cpool,
            tc.tile_pool(name="g", bufs=6) as gpool,
            tc.tile_pool(name="s", bufs=6) as spool,
            tc.tile_pool(name="hsb", bufs=4) as hsb_pool,
            tc.tile_pool(name="htsb", bufs=4) as htsb_pool,
            tc.tile_pool(name="osb", bufs=2) as osb_pool,
            tc.tile_pool(name="hps", bufs=2, space="PSUM") as hps_pool,
            tc.tile_pool(name="htps", bufs=2, space="PSUM") as htps_pool,
            tc.tile_pool(name="ops", bufs=2, space="PSUM") as ops_pool,
        ):
            idx_sb = cpool.tile([128, total_blk * 8], dt.int16)
            nc.sync.dma_start(idx_sb[:], idxd[:])
            w_sb = cpool.tile([128, 16 * OUT_DIM], dt.bfloat16)
            nc.sync.dma_start(w_sb[:], wd[:])
            bm_sb = cpool.tile([128, OUT_DIM], dt.float32)
            nc.sync.dma_start(bm_sb[:], bmd[:])
            ident = cpool.tile([128, 128], dt.bfloat16)
            nc.sync.dma_start(ident[:], identd[:])

            off = 0
            for t in range(NTILES):
                out_ps = ops_pool.tile([128, OUT_DIM], dt.float32)
                for p in range(N_PATHS):
                    call = t * N_PATHS + p
                    Bc = blocks[call]
                    g = gpool.tile([128, Bc, IN_DIM], dt.bfloat16)
                    # split the gather into sub-calls of <= SPLIT_BLOCKS blocks
                    for j0 in range(0, Bc, SPLIT_BLOCKS):
                        j1 = min(j0 + SPLIT_BLOCKS, Bc)
                        nb = j1 - j0
                        nc.gpsimd.dma_gather(
                            g[:, j0:j1, :],
                            featd[:],
                            idx_sb[:, (off + j0) * 8 : (off + j1) * 8],
                            nb * 128,
                            nb * 128,
                            IN_DIM,
                            single_packet=False,
                            queue_num=next_q(),
                        )
                    S = spool.tile([128, Bc * 128], dt.bfloat16)
                    nc.sync.dma_start(S[:], sd[:, off * 128 : (off + Bc) * 128])
                    hp = hps_pool.tile([128, IN_DIM], dt.float32)
                    for bb in range(Bc):
                        nc.tensor.matmul(
                            hp[:],
                            S[:, bb * 128 : (bb + 1) * 128],
                            g[:, bb, :],
                            start=(bb == 0),
                            stop=(bb == Bc - 1),
                        )
                    hs = hsb_pool.tile([128, IN_DIM], dt.bfloat16)
                    nc.scalar.copy(hs[:], hp[:])
                    htp = htps_pool.tile([128, IN_DIM], dt.bfloat16)
                    for cc in range(4):
                        nc.tensor.transpose(
                            htp[:, cc * 128 : (cc + 1) * 128],
                            hs[:, cc * 128 : (cc + 1) * 128],
                            ident[:],
                        )
                    hts = htsb_pool.tile([128, IN_DIM], dt.bfloat16)
                    nc.vector.tensor_copy(hts[:], htp[:])
                    for cc in range(4):
                        nc.tensor.matmul(
                            out_ps[:],
                            hts[:, cc * 128 : (cc + 1) * 128],
                            w_sb[:, (p * 4 + cc) * OUT_DIM : (p * 4 + cc + 1) * OUT_DIM],
                            start=(p == 0 and cc == 0),
                            stop=(p == N_PATHS - 1 and cc == 3),
                        )
                    off += Bc
                os_ = osb_pool.tile([128, OUT_DIM], dt.float32)
                nc.vector.tensor_add(os_[:], out_ps[:], bm_sb[:])
                rows = min(128, ROWS_PER_CORE - t * 128)
                nc.sync.dma_start(outd[t * 128 : t * 128 + rows, :], os_[:rows, :])

    nc.compile()
    _program_cache[key] = nc
    return nc


def _prep_host(feat, src, dst, W, b):
    src = np.asarray(src).astype(np.int64)
    dst = np.asarray(dst).astype(np.int64)
    feat = np.asarray(feat, dtype=np.float32)
    W = np.asarray(W, dtype=np.float32)
    b = np.asarray(b, dtype=np.float32)

    feat_bf = feat.astype(BF16)

    Wt = np.empty((128, 16, OUT_DIM), dtype=BF16)
    for p in range(N_PATHS):
        for c in range(4):
            Wt[:, p * 4 + c, :] = W[p, c * 128 : (c + 1) * 128, :].astype(BF16)
    Wt = np.ascontiguousarray(Wt.reshape(128, 16 * OUT_DIM))

    bmean = b.mean(0).astype(np.float32)
    bm_bcast = np.ascontiguousarray(np.broadcast_to(bmean, (128, OUT_DIM)))

    sorted_data = []
    for p in range(N_PATHS):
        s, d = src[p], dst[p]
        deg_out = np.maximum(np.bincount(s, minlength=N_NODES), 1).astype(np.float64)
        deg_in = np.maximum(np.bincount(d, minlength=N_NODES), 1).astype(np.float64)
        ce = (deg_in[d] ** -0.5) * (deg_out[s] ** -0.5) * 0.25
        order = np.argsort(d, kind="stable")
        sorted_data.append((s[order], d[order], ce[order]))

    bounds_lo = np.array(
        [c * ROWS_PER_CORE + t * 128 for c in range(NCORES) for t in range(NTILES)]
    )
    bounds_hi = np.array(
        [
            c * ROWS_PER_CORE + min((t + 1) * 128, ROWS_PER_CORE)
            for c in range(NCORES)
            for t in range(NTILES)
        ]
    )

    ranges = []
    for p in range(N_PATHS):
        ds = sorted_data[p][1]
        a = np.searchsorted(ds, bounds_lo, side="left")
        e = np.searchsorted(ds, bounds_hi, side="left")
        ranges.append((a, e))

    # gather per-core bucket data; dedup if enabled; compute slot counts
    # buckets[c][t][p] = (slot_idx_array, edge_slot_pos, dd, cc)
    buckets = [[[None] * N_PATHS for _ in range(NTILES)] for _ in range(NCORES)]
    slot_counts = np.zeros((NCORES, NTILES, N_PATHS), dtype=np.int64)
    for c in range(NCORES):
        for t in range(NTILES):
            lo = c * ROWS_PER_CORE + t * 128
            for p in range(N_PATHS):
                a, e = ranges[p][0][c * NTILES + t], ranges[p][1][c * NTILES + t]
                ss = sorted_data[p][0][a:e]
                dd = (sorted_data[p][1][a:e] - lo).astype(np.int64)
                cc = sorted_data[p][2][a:e]
                if DEDUP:
                    uniq, inv = np.unique(ss, return_inverse=True)
                    buckets[c][t][p] = (uniq, inv, dd, cc)
                    slot_counts[c, t, p] = len(uniq)
                else:
                    pos = np.arange(len(ss))
                    buckets[c][t][p] = (ss, pos, dd, cc)
                    slot_counts[c, t, p] = len(ss)

    blocks = []
    for t in range(NTILES):
        for p in range(N_PATHS):
            mx = slot_counts[:, t, p].max()
            blocks.append(int(np.ceil(mx / 128)) or 1)
    blocks = tuple(blocks)
    total_blk = sum(blocks)

    per_core = []
    for c in range(NCORES):
        idxw = np.zeros((128, total_blk * 8), dtype=np.int16)
        smat = np.zeros((128, total_blk * 128), dtype=np.float32)
        off = 0
        for t in range(NTILES):
            for p in range(N_PATHS):
                call = t * N_PATHS + p
                Bc = blocks[call]
                uniq, inv, dd, cc = buckets[c][t][p]
                idx_pad = np.zeros(Bc * 128, dtype=np.int16)
                idx_pad[: len(uniq)] = uniq
                w16 = idx_pad.reshape(Bc * 8, 16).T
                idxw[:, off * 8 : (off + Bc) * 8] = np.tile(w16, (8, 1))
                # S[slot % 128, off*128 + (slot//128)*128 + dst_local] += c_e
                np.add.at(
                    smat,
                    (inv % 128, off * 128 + (inv // 128) * 128 + dd),
                    cc.astype(np.float32),
                )
                off += Bc
        per_core.append({"idx": idxw, "smat": smat.astype(BF16)})

    shared = {
        "feat": feat_bf,
        "w": Wt,
        "bm": bm_bcast,
        "identity": np.eye(128, dtype=BF16),
    }
    return blocks, shared, per_core


def kernel(feat, src, dst, W, b):
    blocks, shared, per_core = _prep_host(feat, src, dst, W, b)
    nc = _build_program(blocks)
    in_maps = [{**shared, **pc} for pc in per_core]
    res = run_bass_kernel_spmd(nc, in_maps, list(range(NCORES)))
    out = np.concatenate([res.results[c]["out"] for c in range(NCORES)], axis=0)
    return out.astype(np.float32)


if __name__ == "__main__":
    rng = np.random.default_rng(0)
    feat = rng.standard_normal((N_NODES, IN_DIM), dtype=np.float32)
    src = rng.integers(0, N_NODES, (N_PATHS, 160000)).astype(np.int64)
    dst = rng.integers(0, N_NODES, (N_PATHS, 160000)).astype(np.int64)
    W = (rng.standard_normal((N_PATHS, IN_DIM, OUT_DIM), dtype=np.float32) / np.sqrt(IN_DIM)).astype(np.float32)
    b = np.zeros((N_PATHS, OUT_DIM), np.float32)
    out = kernel(feat=feat, src=src, dst=dst, W=W, b=b)
    print("kernel ran, out shape", out.shape, out.dtype)
